# revision 25
# baseline (speedup 1.0000x reference)
"""Trainium2 Bass kernel: depthwise (per-sample, per-channel) 15x15 'same'
true convolution of 1024x3 images of 128x128, data-parallel over 8 NeuronCores.

Formulation (per (bn,c) pair, P=128, K=15, pad=7):
    out[y,x] = sum_{dy,dx} Xp[y+dy, x+dx] * Wf[dy,dx],   Wf = flip(kernel),
    Xp = zero-padded image [142, 143].
Output rows are split into 4 blocks of 32 (j in 0..31). Each block runs on its
own 32-wide column strip of the PE array (tile_position=(0, 32b)) so the four
blocks' matmuls execute concurrently. Contraction (92) packs two dx taps:
segment s in {0,1} holds image rows 32b..32b+45 shifted s columns. Pass t
(t=0..7) covers dx = 2t+s via a moving-operand column offset of 2t;
the stationary Toeplitz slab T[46s+i, t, j] = Wf[i-j, 2t+s] accumulates all 8
passes into PSUM [32, 128] per block.

Data staging (per group of G=8 pairs): images are stored pair-interleaved in
DRAM ([row][pair][143]) so one DMA with 2288-byte runs fills segment 0 of the
x4 tile ([92, 4*G*143]); segment 1 (shift-by-one-column copy) is an
SBUF->SBUF DMA with one descriptor per partition. Toeplitz slabs and fp16
outputs are similarly group-batched. Sharding: pure data parallel over BN
(128 samples x 3 channels = 384 pairs per core).
"""
import sys

sys.path.insert(0, "/opt/trn_rl_repo")

import numpy as np

_N_CORES = 8
_BN, _C, _P, _K = 1024, 3, 128, 15
_PAIRS_PER_CORE = (_BN // _N_CORES) * _C  # 384
_G = 32                      # pairs per DMA group
_NG = _PAIRS_PER_CORE // _G  # 48
_XW = 143                    # padded image width (cols 0..142)
_XH = 142                    # padded image height
_ROWP = _G * _XW             # elems per padded row across a group (1144)
_GRP = _XH * _ROWP           # elems per group image block (162448)
_SLAB = 4 * _ROWP            # x4 tile free elems (4576) + 1 slack

_nc_cache = {}


def _build_nc(bufs: int = 3, psum_bufs: int = 4, ot_bufs: int = 2):
    import concourse.bacc as bacc
    import concourse.mybir as mybir
    from concourse import bass, tile

    FP16 = mybir.dt.float16
    FP32 = mybir.dt.float32

    nc = bacc.Bacc("TRN2", target_bir_lowering=False, debug=False)
    xpad_d = nc.dram_tensor("xpad", [_NG * _GRP + 64], FP16, kind="ExternalInput")
    toep_d = nc.dram_tensor("toep", [_NG, 92, _G * 8 * 32], FP16, kind="ExternalInput")
    out_d = nc.dram_tensor("out", [_NG, 128, _G * 256], FP16, kind="ExternalOutput")

    with tile.TileContext(nc) as tc:
        with (
            tc.tile_pool(name="x4", bufs=bufs) as x4_pool,
            tc.tile_pool(name="tt", bufs=bufs) as tt_pool,
            tc.tile_pool(name="ot", bufs=ot_bufs) as ot_pool,
            tc.tile_pool(name="ps", bufs=psum_bufs, space="PSUM") as ps_pool,
        ):
            def emit_loads(grp):
                # loads go on the gpsimd SWDGE queue; keeping them ahead of
                # the previous group's out-DMA in program order (software
                # pipelining below) stops the gen FIFO from serializing
                # group boundaries
                x4 = x4_pool.tile([92, _SLAB + 1], FP16, tag="x4", name="x4")
                tt = tt_pool.tile([92, _G * 8 * 32], FP16, tag="tt", name="tt")
                src0 = bass.AP(
                    tensor=xpad_d.tensor if hasattr(xpad_d, "tensor") else xpad_d,
                    offset=grp * _GRP,
                    ap=[[_ROWP, 46], [32 * _ROWP, 4], [_XW, _G], [1, _XW]],
                )
                nc.gpsimd.dma_start(out=x4[0:46, 0:_SLAB], in_=src0)
                # seg1: partitions 46..91 <- same, shifted one column
                nc.gpsimd.dma_start(out=x4[46:92, 0:_SLAB], in_=x4[0:46, 1:_SLAB + 1])
                nc.sync.dma_start(out=tt[0:46, :], in_=toep_d[grp, 0:46])
                nc.scalar.dma_start(out=tt[46:92, :], in_=toep_d[grp, 46:92])
                return x4, tt

            staged = [emit_loads(0), emit_loads(1)]
            for grp in range(_NG):
                x4, tt = staged.pop(0)
                ot = ot_pool.tile([128, _G * 256], FP16, tag="ot")

                x4ap = x4[:]
                pitch = _SLAB + 1
                for g in range(_G):
                    psA = ps_pool.tile([128, 512], FP32, tag="psA")
                    psB = ps_pool.tile([128, 512], FP32, tag="psB")
                    for t in range(8):
                        par = t & 1
                        for h in range(2):
                            s = 2 * par + h
                            rhs = bass.AP(
                                tensor=x4ap.tensor,
                                offset=x4ap.offset
                                + (2 * h * _G + g) * _XW + 2 * t,
                                ap=[[pitch, 92], [_G * _XW, 2], [1, 128]],
                            )
                            out_ps = (psA[32 * s:32 * s + 32, 0:256] if par == 0
                                      else psB[32 * s:32 * s + 32, 0:256])
                            nc.tensor.matmul(
                                out_ps,
                                tt[0:92, (g * 8 + t) * 32:(g * 8 + t) * 32 + 32],
                                rhs,
                                start=(t < 2), stop=(t >= 6),
                                tile_position=(0, 32 * s),
                            )
                    nc.vector.tensor_copy(
                        ot[0:64, g * 256:(g + 1) * 256], psA[0:64, 0:256])
                    nc.scalar.copy(
                        ot[64:128, g * 256:(g + 1) * 256], psB[64:128, 0:256])

                if grp + 2 < _NG:
                    staged.append(emit_loads(grp + 2))
                nc.gpsimd.dma_start(out=out_d[grp], in_=ot[:])

    nc.compile()
    return nc


def _host_prep(patches_pairs: np.ndarray, kernels_pairs: np.ndarray):
    """[NP,128,128] f32, [NP,15,15] f32 -> (xpad flat fp16, toep fp16).

    xpad: [NG*142*G*143 + 64] with layout [grp][row 142][pair G][col 143],
    zero-padded images at rows/cols 7..134.
    toep: [NG, 92, G*8*32] with T[p][46s+i, t, j] = Wf[i-j, 2t+s]
    (0 <= i-j < 15, dx = 2t+s <= 14), layout [grp][i_stack][pair][t][j].
    """
    NP = patches_pairs.shape[0]
    assert NP == _PAIRS_PER_CORE
    Xp = np.zeros((_NG, _G, _XH, _XW), dtype=np.float16)
    Xp[:, :, 7:135, 7:135] = patches_pairs.reshape(_NG, _G, 128, 128)
    xpad = np.zeros(_NG * _GRP + 64, dtype=np.float16)
    xpad[:_NG * _GRP] = np.ascontiguousarray(
        Xp.transpose(0, 2, 1, 3)).reshape(-1)

    Wf = np.ascontiguousarray(
        kernels_pairs[:, ::-1, ::-1]).astype(np.float16)  # [NP, 15, 15]
    T = np.zeros((NP, 2, 46, 8, 32), dtype=np.float16)
    j = np.arange(32)
    for dy in range(15):
        for t in range(8):
            for s in range(2):
                dx = 2 * t + s
                if dx > 14:
                    continue
                T[:, s, j + dy, t, j] = Wf[:, dy, dx][:, None]
    T = T.reshape(_NG, _G, 92, 8 * 32).transpose(0, 2, 1, 3)
    toep = np.ascontiguousarray(T).reshape(_NG, 92, _G * 8 * 32)
    return xpad, toep


def kernel(patches, kernels, kernel_size, patch_size, fft_size, _collect_results=None):
    """Full inputs in, full output out. Shards BN across 8 cores."""
    from concourse.bass_utils import run_bass_kernel_spmd

    patches = np.asarray(patches)
    kernels = np.asarray(kernels)
    assert patches.shape == (_BN, _C, _P, _P), patches.shape
    assert kernels.shape == (_BN, _C, _K, _K), kernels.shape

    if "nc" not in _nc_cache:
        _nc_cache["nc"] = _build_nc()
    nc = _nc_cache["nc"]

    bn_per_core = _BN // _N_CORES
    in_maps = []
    for core in range(_N_CORES):
        sl = slice(core * bn_per_core, (core + 1) * bn_per_core)
        pp = patches[sl].reshape(-1, _P, _P)
        kp = kernels[sl].reshape(-1, _K, _K)
        xpad, toep = _host_prep(pp, kp)
        in_maps.append({"xpad": xpad, "toep": toep})

    res = run_bass_kernel_spmd(nc, in_maps, core_ids=list(range(_N_CORES)))
    if _collect_results is not None:
        _collect_results.append(res)

    out = np.empty((_BN, _C, _P, _P), dtype=np.float32)
    for core in range(_N_CORES):
        sl = slice(core * bn_per_core, (core + 1) * bn_per_core)
        o = res.results[core]["out"].reshape(_NG, 2, 2, 32, _G, 2, 128)
        s = o[:, 0].astype(np.float32) + o[:, 1].astype(np.float32)
        # [NG, 2h, 32j, G, 2db, 128x] -> [NG, G, h, db, j, x] -> [pairs, y, x]
        out[sl] = s.transpose(0, 3, 1, 4, 2, 5).reshape(
            bn_per_core, _C, _P, _P)
    return out


# revision 26
# speedup vs baseline: 1.1992x; 1.1992x over previous
"""Trainium2 Bass kernel: depthwise (per-sample, per-channel) 15x15 'same'
true convolution of 1024x3 images of 128x128, data-parallel over 8 NeuronCores.

Formulation (per (bn,c) pair, P=128, K=15, pad=7):
    out[y,x] = sum_{dy,dx} Xp[y+dy, x+dx] * Wf[dy,dx],   Wf = flip(kernel),
    Xp = zero-padded image [142, 143].
Output rows are split into 4 blocks of 32 (j in 0..31). Each block runs on its
own 32-wide column strip of the PE array (tile_position=(0, 32b)) so the four
blocks' matmuls execute concurrently. Contraction (92) packs two dx taps:
segment s in {0,1} holds image rows 32b..32b+45 shifted s columns. Pass t
(t=0..7) covers dx = 2t+s via a moving-operand column offset of 2t;
the stationary Toeplitz slab T[46s+i, t, j] = Wf[i-j, 2t+s] accumulates all 8
passes into PSUM [32, 128] per block.

Data staging (per group of G=8 pairs): images are stored pair-interleaved in
DRAM ([row][pair][143]) so one DMA with 2288-byte runs fills segment 0 of the
x4 tile ([92, 4*G*143]); segment 1 (shift-by-one-column copy) is an
SBUF->SBUF DMA with one descriptor per partition. Toeplitz slabs and fp16
outputs are similarly group-batched. Sharding: pure data parallel over BN
(128 samples x 3 channels = 384 pairs per core).
"""
import sys

sys.path.insert(0, "/opt/trn_rl_repo")

import numpy as np

_N_CORES = 8
_BN, _C, _P, _K = 1024, 3, 128, 15
_PAIRS_PER_CORE = (_BN // _N_CORES) * _C  # 384
_G = 32                      # pairs per DMA group
_NG = _PAIRS_PER_CORE // _G  # 48
_XW = 143                    # padded image width (cols 0..142)
_XH = 142                    # padded image height
_ROWP = _G * _XW             # elems per padded row across a group (1144)
_GRP = _XH * _ROWP           # elems per group image block (162448)
_SLAB = 4 * _ROWP            # x4 tile free elems (4576) + 1 slack

_nc_cache = {}


def _build_nc(bufs: int = 3, psum_bufs: int = 4, ot_bufs: int = 2):
    import concourse.bacc as bacc
    import concourse.mybir as mybir
    from concourse import bass, tile

    FP16 = mybir.dt.float16
    FP32 = mybir.dt.float32

    nc = bacc.Bacc("TRN2", target_bir_lowering=False, debug=False)
    xpad_d = nc.dram_tensor("xpad", [_NG * _GRP + 64], FP16, kind="ExternalInput")
    toep_d = nc.dram_tensor("toep", [_NG, 92, _G * 8 * 32], FP16, kind="ExternalInput")
    out_d = nc.dram_tensor("out", [_NG, 128, _G * 128], FP16, kind="ExternalOutput")

    with tile.TileContext(nc) as tc:
        with (
            tc.tile_pool(name="x4", bufs=bufs) as x4_pool,
            tc.tile_pool(name="tt", bufs=bufs) as tt_pool,
            tc.tile_pool(name="ot", bufs=ot_bufs) as ot_pool,
            tc.tile_pool(name="ps", bufs=psum_bufs, space="PSUM") as ps_pool,
        ):
            def emit_loads(grp):
                # loads go on the gpsimd SWDGE queue; keeping them ahead of
                # the previous group's out-DMA in program order (software
                # pipelining below) stops the gen FIFO from serializing
                # group boundaries
                x4 = x4_pool.tile([92, _SLAB + 1], FP16, tag="x4", name="x4")
                tt = tt_pool.tile([92, _G * 8 * 32], FP16, tag="tt", name="tt")
                src0 = bass.AP(
                    tensor=xpad_d.tensor if hasattr(xpad_d, "tensor") else xpad_d,
                    offset=grp * _GRP,
                    ap=[[_ROWP, 46], [32 * _ROWP, 4], [_XW, _G], [1, _XW]],
                )
                nc.gpsimd.dma_start(out=x4[0:46, 0:_SLAB], in_=src0)
                # seg1: partitions 46..91 <- same, shifted one column
                nc.gpsimd.dma_start(out=x4[46:92, 0:_SLAB], in_=x4[0:46, 1:_SLAB + 1])
                nc.gpsimd.dma_start(out=tt[:], in_=toep_d[grp])
                return x4, tt

            staged = [emit_loads(0), emit_loads(1)]
            for grp in range(_NG):
                x4, tt = staged.pop(0)
                ot = ot_pool.tile([128, _G * 128], FP16, tag="ot")

                for g in range(_G):
                    ps = ps_pool.tile([128, 128], FP32, tag="ps")
                    for t in range(8):
                        for b in range(4):
                            nc.tensor.matmul(
                                ps[32 * b:32 * b + 32, :],
                                tt[0:92, (g * 8 + t) * 32:(g * 8 + t) * 32 + 32],
                                x4[0:92, (b * _G + g) * _XW + 2 * t:
                                   (b * _G + g) * _XW + 2 * t + 128],
                                start=(t == 0), stop=(t == 7),
                                tile_position=(0, 32 * b),
                            )
                    nc.vector.tensor_copy(
                        ot[0:64, g * 128:(g + 1) * 128], ps[0:64, :])
                    nc.scalar.copy(
                        ot[64:128, g * 128:(g + 1) * 128], ps[64:128, :])

                if grp + 2 < _NG:
                    staged.append(emit_loads(grp + 2))
                nc.gpsimd.dma_start(out=out_d[grp], in_=ot[:])

    nc.compile()
    return nc


def _host_prep(patches_pairs: np.ndarray, kernels_pairs: np.ndarray):
    """[NP,128,128] f32, [NP,15,15] f32 -> (xpad flat fp16, toep fp16).

    xpad: [NG*142*G*143 + 64] with layout [grp][row 142][pair G][col 143],
    zero-padded images at rows/cols 7..134.
    toep: [NG, 92, G*8*32] with T[p][46s+i, t, j] = Wf[i-j, 2t+s]
    (0 <= i-j < 15, dx = 2t+s <= 14), layout [grp][i_stack][pair][t][j].
    """
    NP = patches_pairs.shape[0]
    assert NP == _PAIRS_PER_CORE
    Xp = np.zeros((_NG, _G, _XH, _XW), dtype=np.float16)
    Xp[:, :, 7:135, 7:135] = patches_pairs.reshape(_NG, _G, 128, 128)
    xpad = np.zeros(_NG * _GRP + 64, dtype=np.float16)
    xpad[:_NG * _GRP] = np.ascontiguousarray(
        Xp.transpose(0, 2, 1, 3)).reshape(-1)

    Wf = np.ascontiguousarray(
        kernels_pairs[:, ::-1, ::-1]).astype(np.float16)  # [NP, 15, 15]
    T = np.zeros((NP, 2, 46, 8, 32), dtype=np.float16)
    j = np.arange(32)
    for dy in range(15):
        for t in range(8):
            for s in range(2):
                dx = 2 * t + s
                if dx > 14:
                    continue
                T[:, s, j + dy, t, j] = Wf[:, dy, dx][:, None]
    T = T.reshape(_NG, _G, 92, 8 * 32).transpose(0, 2, 1, 3)
    toep = np.ascontiguousarray(T).reshape(_NG, 92, _G * 8 * 32)
    return xpad, toep


def kernel(patches, kernels, kernel_size, patch_size, fft_size, _collect_results=None):
    """Full inputs in, full output out. Shards BN across 8 cores."""
    from concourse.bass_utils import run_bass_kernel_spmd

    patches = np.asarray(patches)
    kernels = np.asarray(kernels)
    assert patches.shape == (_BN, _C, _P, _P), patches.shape
    assert kernels.shape == (_BN, _C, _K, _K), kernels.shape

    if "nc" not in _nc_cache:
        _nc_cache["nc"] = _build_nc()
    nc = _nc_cache["nc"]

    bn_per_core = _BN // _N_CORES
    in_maps = []
    for core in range(_N_CORES):
        sl = slice(core * bn_per_core, (core + 1) * bn_per_core)
        pp = patches[sl].reshape(-1, _P, _P)
        kp = kernels[sl].reshape(-1, _K, _K)
        xpad, toep = _host_prep(pp, kp)
        in_maps.append({"xpad": xpad, "toep": toep})

    res = run_bass_kernel_spmd(nc, in_maps, core_ids=list(range(_N_CORES)))
    if _collect_results is not None:
        _collect_results.append(res)

    out = np.empty((_BN, _C, _P, _P), dtype=np.float32)
    for core in range(_N_CORES):
        sl = slice(core * bn_per_core, (core + 1) * bn_per_core)
        o = res.results[core]["out"].reshape(_NG, 128, _G, 128)
        out[sl] = o.transpose(0, 2, 1, 3).reshape(
            bn_per_core, _C, _P, _P).astype(np.float32)
    return out


# revision 28
# speedup vs baseline: 1.2185x; 1.0161x over previous
"""Trainium2 Bass kernel: depthwise (per-sample, per-channel) 15x15 'same'
true convolution of 1024x3 images of 128x128, data-parallel over 8 NeuronCores.

Formulation (per (bn,c) pair, P=128, K=15, pad=7):
    out[y,x] = sum_{dy,dx} Xp[y+dy, x+dx] * Wf[dy,dx],   Wf = flip(kernel),
    Xp = zero-padded image [142, 143].
Output rows are split into 4 blocks of 32 (j in 0..31). Each block runs on its
own 32-wide column strip of the PE array (tile_position=(0, 32b)) so the four
blocks' matmuls execute concurrently. Contraction (92) packs two dx taps:
segment s in {0,1} holds image rows 32b..32b+45 shifted s columns. Pass t
(t=0..7) covers dx = 2t+s via a moving-operand column offset of 2t;
the stationary Toeplitz slab T[46s+i, t, j] = Wf[i-j, 2t+s] accumulates all 8
passes into PSUM [32, 128] per block.

Data staging (per group of G=8 pairs): images are stored pair-interleaved in
DRAM ([row][pair][143]) so one DMA with 2288-byte runs fills segment 0 of the
x4 tile ([92, 4*G*143]); segment 1 (shift-by-one-column copy) is an
SBUF->SBUF DMA with one descriptor per partition. Toeplitz slabs and fp16
outputs are similarly group-batched. Sharding: pure data parallel over BN
(128 samples x 3 channels = 384 pairs per core).
"""
import sys

sys.path.insert(0, "/opt/trn_rl_repo")

import numpy as np

_N_CORES = 8
_BN, _C, _P, _K = 1024, 3, 128, 15
_PAIRS_PER_CORE = (_BN // _N_CORES) * _C  # 384
_G = 24                      # pairs per DMA group
_NG = _PAIRS_PER_CORE // _G  # 48
_XW = 143                    # padded image width (cols 0..142)
_XH = 142                    # padded image height
_ROWP = _G * _XW             # elems per padded row across a group (1144)
_GRP = _XH * _ROWP           # elems per group image block (162448)
_SLAB = 4 * _ROWP            # x4 tile free elems (4576) + 1 slack

_nc_cache = {}


def _build_nc(bufs: int = 4, psum_bufs: int = 4, ot_bufs: int = 3):
    import concourse.bacc as bacc
    import concourse.mybir as mybir
    from concourse import bass, tile

    FP16 = mybir.dt.float16
    FP32 = mybir.dt.float32

    nc = bacc.Bacc("TRN2", target_bir_lowering=False, debug=False)
    xpad_d = nc.dram_tensor("xpad", [_NG * _GRP + 64], FP16, kind="ExternalInput")
    toep_d = nc.dram_tensor("toep", [_NG, 92, _G * 8 * 32], FP16, kind="ExternalInput")
    out_d = nc.dram_tensor("out", [_NG, 128, _G * 256], FP16, kind="ExternalOutput")

    with tile.TileContext(nc) as tc:
        with (
            tc.tile_pool(name="x4", bufs=bufs) as x4_pool,
            tc.tile_pool(name="tt", bufs=bufs) as tt_pool,
            tc.tile_pool(name="ot", bufs=ot_bufs) as ot_pool,
            tc.tile_pool(name="ps", bufs=psum_bufs, space="PSUM") as ps_pool,
        ):
            def emit_loads(grp):
                # loads go on the gpsimd SWDGE queue; keeping them ahead of
                # the previous group's out-DMA in program order (software
                # pipelining below) stops the gen FIFO from serializing
                # group boundaries
                x4 = x4_pool.tile([92, _SLAB + 1], FP16, tag="x4", name="x4")
                tt = tt_pool.tile([92, _G * 8 * 32], FP16, tag="tt", name="tt")
                src0 = bass.AP(
                    tensor=xpad_d.tensor if hasattr(xpad_d, "tensor") else xpad_d,
                    offset=grp * _GRP,
                    ap=[[_ROWP, 46], [32 * _ROWP, 4], [_XW, _G], [1, _XW]],
                )
                nc.gpsimd.dma_start(out=x4[0:46, 0:_SLAB], in_=src0)
                # seg1: partitions 46..91 <- same, shifted one column
                nc.gpsimd.dma_start(out=x4[46:92, 0:_SLAB], in_=x4[0:46, 1:_SLAB + 1])
                nc.gpsimd.dma_start(out=tt[:], in_=toep_d[grp])
                return x4, tt

            staged = [emit_loads(0), emit_loads(1), emit_loads(2)]
            for grp in range(_NG):
                x4, tt = staged.pop(0)
                ot = ot_pool.tile([128, _G * 256], FP16, tag="ot")

                x4ap = x4[:]
                pitch = _SLAB + 1
                for g in range(_G):
                    psA = ps_pool.tile([128, 512], FP32, tag="psA")
                    psB = ps_pool.tile([128, 512], FP32, tag="psB")
                    for t in range(8):
                        par = t & 1
                        for h in range(2):
                            s = 2 * par + h
                            rhs = bass.AP(
                                tensor=x4ap.tensor,
                                offset=x4ap.offset
                                + (2 * h * _G + g) * _XW + 2 * t,
                                ap=[[pitch, 92], [_G * _XW, 2], [1, 128]],
                            )
                            out_ps = (psA[32 * s:32 * s + 32, 0:256] if par == 0
                                      else psB[32 * s:32 * s + 32, 0:256])
                            nc.tensor.matmul(
                                out_ps,
                                tt[0:92, (g * 8 + t) * 32:(g * 8 + t) * 32 + 32],
                                rhs,
                                start=(t < 2), stop=(t >= 6),
                                tile_position=(0, 32 * s),
                            )
                    nc.vector.tensor_copy(
                        ot[0:64, g * 256:(g + 1) * 256], psA[0:64, 0:256])
                    nc.scalar.copy(
                        ot[64:128, g * 256:(g + 1) * 256], psB[64:128, 0:256])

                if grp + 3 < _NG:
                    staged.append(emit_loads(grp + 3))
                nc.gpsimd.dma_start(out=out_d[grp], in_=ot[:])

    nc.compile()
    return nc


def _host_prep(patches_pairs: np.ndarray, kernels_pairs: np.ndarray):
    """[NP,128,128] f32, [NP,15,15] f32 -> (xpad flat fp16, toep fp16).

    xpad: [NG*142*G*143 + 64] with layout [grp][row 142][pair G][col 143],
    zero-padded images at rows/cols 7..134.
    toep: [NG, 92, G*8*32] with T[p][46s+i, t, j] = Wf[i-j, 2t+s]
    (0 <= i-j < 15, dx = 2t+s <= 14), layout [grp][i_stack][pair][t][j].
    """
    NP = patches_pairs.shape[0]
    assert NP == _PAIRS_PER_CORE
    Xp = np.zeros((_NG, _G, _XH, _XW), dtype=np.float16)
    Xp[:, :, 7:135, 7:135] = patches_pairs.reshape(_NG, _G, 128, 128)
    xpad = np.zeros(_NG * _GRP + 64, dtype=np.float16)
    xpad[:_NG * _GRP] = np.ascontiguousarray(
        Xp.transpose(0, 2, 1, 3)).reshape(-1)

    Wf = np.ascontiguousarray(
        kernels_pairs[:, ::-1, ::-1]).astype(np.float16)  # [NP, 15, 15]
    T = np.zeros((NP, 2, 46, 8, 32), dtype=np.float16)
    j = np.arange(32)
    for dy in range(15):
        for t in range(8):
            for s in range(2):
                dx = 2 * t + s
                if dx > 14:
                    continue
                T[:, s, j + dy, t, j] = Wf[:, dy, dx][:, None]
    T = T.reshape(_NG, _G, 92, 8 * 32).transpose(0, 2, 1, 3)
    toep = np.ascontiguousarray(T).reshape(_NG, 92, _G * 8 * 32)
    return xpad, toep


def kernel(patches, kernels, kernel_size, patch_size, fft_size, _collect_results=None):
    """Full inputs in, full output out. Shards BN across 8 cores."""
    from concourse.bass_utils import run_bass_kernel_spmd

    patches = np.asarray(patches)
    kernels = np.asarray(kernels)
    assert patches.shape == (_BN, _C, _P, _P), patches.shape
    assert kernels.shape == (_BN, _C, _K, _K), kernels.shape

    if "nc" not in _nc_cache:
        _nc_cache["nc"] = _build_nc()
    nc = _nc_cache["nc"]

    bn_per_core = _BN // _N_CORES
    in_maps = []
    for core in range(_N_CORES):
        sl = slice(core * bn_per_core, (core + 1) * bn_per_core)
        pp = patches[sl].reshape(-1, _P, _P)
        kp = kernels[sl].reshape(-1, _K, _K)
        xpad, toep = _host_prep(pp, kp)
        in_maps.append({"xpad": xpad, "toep": toep})

    res = run_bass_kernel_spmd(nc, in_maps, core_ids=list(range(_N_CORES)))
    if _collect_results is not None:
        _collect_results.append(res)

    out = np.empty((_BN, _C, _P, _P), dtype=np.float32)
    for core in range(_N_CORES):
        sl = slice(core * bn_per_core, (core + 1) * bn_per_core)
        o = res.results[core]["out"].reshape(_NG, 2, 2, 32, _G, 2, 128)
        s = o[:, 0].astype(np.float32) + o[:, 1].astype(np.float32)
        # [NG, 2h, 32j, G, 2db, 128x] -> [NG, G, h, db, j, x] -> [pairs, y, x]
        out[sl] = s.transpose(0, 3, 1, 4, 2, 5).reshape(
            bn_per_core, _C, _P, _P)
    return out
